# revision 9
# baseline (speedup 1.0000x reference)
"""Bahdanau attention Trainium2 kernel (8 NeuronCores, data-parallel on batch).

Reference computation (B=32, T=512, DH=2048, fp32):
    query = dec @ Ws_w.T + Ws_b          (t == 0)
    q     = query @ Wa_w.T + Wa_b                     # [B, DH]   (tiny -> host)
    pk    = keys @ Ua_w.T + Ua_b                      # [B, T, DH] (the 137 GFLOP matmul)
    e     = tanh(q[:, None, :] + pk)
    scores= e . Va                                    # [B, T]  (+Va_b: softmax-invariant, dropped)
    attn  = softmax(scores, axis=1)
    av    = attn @ keys                               # [B, DH]

Device strategy per core (4 batches/core):
    - keys^T [DH, 4*T] and Ua^T [DH, DH] fed transposed+bf16 from host
      (contraction dim on partitions; PE needs both operands K-major).
    - main matmul: out[e_chunk(128), t(512)] accumulating 16 k-chunks in PSUM.
    - ACT: tanh(psum + bias) fused, bias = (q+Ua_b)^T per-partition column.
    - scores: PE matmuls lhsT=Va chunk [128,1] accumulated over 16 e-chunks.
    - softmax on [1, 512]: DVE max(negate) -> ACT exp(bias=-max, accum=sum)
      -> DVE reciprocal -> DVE scale.
    - replicate attn to 128 partitions via PE (ones[1,128]^T @ attn[1,512]).
    - av: fused DVE tensor_tensor_reduce over keysT tiles -> av^T [128,16],
      PE-transpose -> [16,128] -> DMA out.
"""

import numpy as np
import ml_dtypes

import concourse.bacc as bacc
import concourse.mybir as mybir
import concourse.tile as tile
from concourse.masks import make_identity
from concourse.bass_utils import run_bass_kernel_spmd

B, T, DH = 32, 512, 2048
NCORES = 8
BPC = B // NCORES          # batches per core
P = 128
KC = DH // P               # contraction chunks
EC = DH // P               # output-feature chunks

F32 = mybir.dt.float32
BF16 = mybir.dt.bfloat16
MM_DT = BF16               # dtype of the big matmuls (bf16: full PE rate)

WARMUP_TABLES = True       # preload ACT Tanh/Exp tables during DMA prologue
PROLOGUE_SPLIT = 5         # batch-0 e-chunks computed k-outer to overlap DMA
INTERLEAVE_VA = True       # emit score matmul for chunk e after main chunk e+1

_nbf = ml_dtypes.bfloat16


def build_program(loop_n=None):
    """Build the per-core Bass program. loop_n wraps the body in a hardware
    loop (timing harness only); None = single-shot (grading path)."""
    nc = bacc.Bacc("TRN2", target_bir_lowering=False, debug=False,
                   num_devices=NCORES)

    keysT = nc.dram_tensor("keysT", [DH, BPC * T], MM_DT, kind="ExternalInput")
    uaT = nc.dram_tensor("uaT", [DH, DH], MM_DT, kind="ExternalInput")
    qbT = nc.dram_tensor("qbT", [P, EC * BPC], F32, kind="ExternalInput")
    vaT = nc.dram_tensor("vaT", [P, EC], MM_DT, kind="ExternalInput")
    av_out = nc.dram_tensor("av_out", [BPC, DH], F32, kind="ExternalOutput")
    attn_out = nc.dram_tensor("attn_out", [BPC, T], F32, kind="ExternalOutput")

    av_dram = av_out.ap().rearrange("b (c p) -> b c p", c=EC)  # [BPC, 16, 128]

    with tile.TileContext(nc) as tc:
        with (
            tc.tile_pool(name="const", bufs=1) as const_pool,
            tc.tile_pool(name="ua", bufs=KC) as ua_pool,
            tc.tile_pool(name="kt", bufs=2 * KC) as kt_pool,
            tc.tile_pool(name="th", bufs=KC + 4) as th_pool,
            tc.tile_pool(name="small", bufs=8) as small_pool,
            tc.tile_pool(name="scr", bufs=2) as scr_pool,
            tc.tile_pool(name="ps_pk", bufs=max(4, PROLOGUE_SPLIT),
                         space="PSUM") as ps_pk_pool,
            tc.tile_pool(name="ps_sc", bufs=1, space="PSUM") as ps_sc_pool,
            tc.tile_pool(name="ps_rep", bufs=1, space="PSUM") as ps_rep_pool,
            tc.tile_pool(name="ps_av", bufs=1, space="PSUM") as ps_av_pool,
        ):
            identity = const_pool.tile([P, P], F32)
            make_identity(nc, identity[:, :])
            ones = const_pool.tile([1, P], F32)
            nc.vector.memset(ones[:, :], 1.0)

            def body():
                if WARMUP_TABLES:
                    # touch Tanh/Exp so ACT table loads overlap the DMA
                    # prologue instead of stalling the first real activation
                    w_ = small_pool.tile([1, 2], F32, tag="warm")
                    nc.vector.memset(w_[:, :], 0.0)
                    nc.scalar.activation(w_[:, :], w_[:, :],
                                         mybir.ActivationFunctionType.Tanh)
                    nc.scalar.activation(w_[:, :], w_[:, :],
                                         mybir.ActivationFunctionType.Exp)

                # static operands (re-fetched per iteration in loop mode; the
                # DMA overlaps compute either way). Interleave the Ua chunks
                # with batch-0 keysT chunks so pass-A matmuls start early.
                ua_sb = []
                kt0 = []
                for k in range(KC):
                    t_ = ua_pool.tile([P, DH], MM_DT, tag="ua")
                    nc.sync.dma_start(t_[:, :], uaT.ap()[k * P:(k + 1) * P, :])
                    ua_sb.append(t_)
                    t_ = kt_pool.tile([P, T], MM_DT, tag="kt")
                    nc.sync.dma_start(t_[:, :],
                                      keysT.ap()[k * P:(k + 1) * P, 0:T])
                    kt0.append(t_)
                qb_sb = small_pool.tile([P, EC * BPC], F32, tag="qb")
                nc.sync.dma_start(qb_sb[:, :], qbT.ap())
                va_sb = small_pool.tile([P, EC], MM_DT, tag="va")
                nc.sync.dma_start(va_sb[:, :], vaT.ap())

                for b in range(BPC):
                    # keys^T tiles for this batch: [128(d), 512(t)] x KC
                    if b == 0:
                        kt = kt0
                    else:
                        kt = []
                        for k in range(KC):
                            t_ = kt_pool.tile([P, T], MM_DT, tag="kt")
                            nc.sync.dma_start(
                                t_[:, :],
                                keysT.ap()[k * P:(k + 1) * P,
                                           b * T:(b + 1) * T])
                            kt.append(t_)

                    th = [None] * EC
                    ps_sc = ps_sc_pool.tile([1, T], F32, tag="ps_sc")
                    n_va = [0]

                    def emit_tanh(e, ps, b=b, th=th):
                        t_ = th_pool.tile([P, T], MM_DT, tag="th")
                        nc.scalar.activation(
                            t_[:, :], ps[:, :],
                            mybir.ActivationFunctionType.Tanh,
                            bias=qb_sb[:, e * BPC + b:e * BPC + b + 1],
                            scale=1.0)
                        th[e] = t_

                    def emit_va(e, ps_sc=ps_sc, th=th, n_va=n_va):
                        # scores[1,512] += Va_chunk[e]^T @ th[e]
                        nc.tensor.matmul(
                            ps_sc[:, :], va_sb[:, e:e + 1], th[e][:, :],
                            start=(n_va[0] == 0), stop=(n_va[0] == EC - 1))
                        n_va[0] += 1

                    # pk[e] = sum_k UaT[k,e]^T @ keysT[k]  -> tanh(+bias)
                    pa = PROLOGUE_SPLIT if b == 0 else 0
                    if pa:
                        # k-outer over the first `pa` e-chunks: the PE chews
                        # each (ua[k], kt0[k]) pair as it lands instead of
                        # stalling until the whole 10MB prologue arrives
                        pss = [ps_pk_pool.tile([P, T], F32, tag="ps_pk",
                                               name=f"pss{i}")
                               for i in range(pa)]
                        for k in range(KC):
                            for e in range(pa):
                                nc.tensor.matmul(
                                    pss[e][:, :],
                                    ua_sb[k][:, e * P:(e + 1) * P],
                                    kt[k][:, :],
                                    start=(k == 0), stop=(k == KC - 1))
                        for e in range(pa):
                            emit_tanh(e, pss[e])

                    va_queue = list(range(pa))
                    for e in range(pa, EC):
                        ps = ps_pk_pool.tile([P, T], F32, tag="ps_pk")
                        for k in range(KC):
                            nc.tensor.matmul(
                                ps[:, :],
                                ua_sb[k][:, e * P:(e + 1) * P],
                                kt[k][:, :],
                                start=(k == 0), stop=(k == KC - 1))
                        emit_tanh(e, ps)
                        if INTERLEAVE_VA:
                            # previous chunk's tanh finished during this
                            # group's matmuls; fold it into scores now
                            while va_queue:
                                emit_va(va_queue.pop(0))
                            va_queue.append(e)
                        else:
                            va_queue.append(e)
                    for e in va_queue:
                        emit_va(e)

                    # softmax over the 512 scores (single partition)
                    negmax = small_pool.tile([1, 1], F32, tag="negmax")
                    nc.vector.tensor_reduce(
                        negmax[:, :], ps_sc[:, :],
                        axis=mybir.AxisListType.X, op=mybir.AluOpType.max,
                        negate=True)
                    exp_sb = small_pool.tile([1, T], F32, tag="exp")
                    sumexp = small_pool.tile([1, 1], F32, tag="sumexp")
                    nc.scalar.activation(
                        exp_sb[:, :], ps_sc[:, :],
                        mybir.ActivationFunctionType.Exp,
                        bias=negmax[:, :], scale=1.0,
                        accum_out=sumexp[:, :])
                    inv = small_pool.tile([1, 1], F32, tag="inv")
                    nc.vector.reciprocal(inv[:, :], sumexp[:, :])
                    attn_sb = small_pool.tile([1, T], F32, tag="attn")
                    nc.vector.tensor_scalar_mul(
                        attn_sb[:, :], exp_sb[:, :], inv[:, :])
                    nc.sync.dma_start(attn_out.ap()[b:b + 1, :], attn_sb[:, :])

                    # replicate attn across partitions: ones^T @ attn
                    ps_rep = ps_rep_pool.tile([P, T], F32, tag="ps_rep")
                    nc.tensor.matmul(ps_rep[:, :], ones[:, :], attn_sb[:, :],
                                     start=True, stop=True)

                    # av^T[:, d] = sum_t keysT[d][:, t] * attn[t]
                    # (tensor_tensor_reduce is broken in this backend --
                    #  use separate multiply + reduce)
                    av_t = small_pool.tile([P, EC], F32, tag="av_t")
                    for d in range(KC):
                        scr = scr_pool.tile([P, T], F32, tag="scr")
                        nc.vector.tensor_tensor(
                            out=scr[:, :], in0=kt[d][:, :], in1=ps_rep[:, :],
                            op=mybir.AluOpType.mult)
                        nc.vector.reduce_sum(
                            av_t[:, d:d + 1], scr[:, :],
                            axis=mybir.AxisListType.X)

                    # transpose [128,16] -> [16,128], copy out
                    ps_av = ps_av_pool.tile([EC, P], F32, tag="ps_av")
                    nc.tensor.transpose(ps_av[:, :], av_t[:, :],
                                        identity[:, :])
                    av_sb = small_pool.tile([EC, P], F32, tag="av_sb")
                    nc.scalar.copy(av_sb[:, :], ps_av[:, :])
                    nc.sync.dma_start(av_dram[b], av_sb[:, :])

            if loop_n is None:
                body()
            else:
                with tc.For_i(0, loop_n, 1,
                              hint_engines=(mybir.EngineType.PE,)):
                    body()

    nc.compile()
    return nc


_CACHE = {}


def _get_program(loop_n=None):
    key = ("prog", loop_n)
    if key not in _CACHE:
        _CACHE[key] = build_program(loop_n)
    return _CACHE[key]


def _make_runner(nc):
    """Build a cached jitted SPMD executor for `nc` (the library's
    run_bass_kernel_spmd re-traces jax.jit on every call, which costs ~3s;
    tracing once makes repeat calls cheap)."""
    import jax
    from jax.experimental.shard_map import shard_map
    from jax.sharding import Mesh, PartitionSpec
    from concourse.bass2jax import (_bass_exec_p, install_neuronx_cc_hook,
                                    partition_id_tensor)

    install_neuronx_cc_hook()

    partition_name = (nc.partition_id_tensor.name
                      if nc.partition_id_tensor else None)
    in_names, out_names, out_avals, zero_shapes = [], [], [], []
    for alloc in nc.m.functions[0].allocations:
        if not isinstance(alloc, mybir.MemoryLocationSet):
            continue
        name = alloc.memorylocations[0].name
        if alloc.kind == "ExternalInput":
            if name != partition_name:
                in_names.append(name)
        elif alloc.kind == "ExternalOutput":
            shape = tuple(alloc.tensor_shape)
            dtype = mybir.dt.np(alloc.dtype)
            out_names.append(name)
            out_avals.append(jax.core.ShapedArray(shape, dtype))
            zero_shapes.append((shape, dtype))
    n_params = len(in_names)
    all_names = list(in_names + out_names)
    if partition_name is not None:
        all_names.append(partition_name)
    all_names = tuple(all_names)
    donate = tuple(range(n_params, n_params + len(out_names)))

    def _body(*args):
        operands = list(args)
        if partition_name is not None:
            operands.append(partition_id_tensor())
        outs = _bass_exec_p.bind(
            *operands,
            out_avals=tuple(out_avals),
            in_names=all_names,
            out_names=tuple(out_names),
            lowering_input_output_aliases=(),
            sim_require_finite=True,
            sim_require_nnan=True,
            nc=nc,
        )
        return tuple(outs)

    devices = jax.devices()[:NCORES]
    mesh = Mesh(np.asarray(devices), ("core",))
    nio = n_params + len(out_names)
    sharded = jax.jit(
        shard_map(_body, mesh=mesh, in_specs=(PartitionSpec("core"),) * nio,
                  out_specs=(PartitionSpec("core"),) * len(out_names),
                  check_rep=False),
        donate_argnums=donate, keep_unused=True)

    def run(in_maps):
        concat_in = [
            np.concatenate([np.asarray(m[name]) for m in in_maps], axis=0)
            for name in in_names
        ]
        concat_zeros = [
            np.zeros((NCORES * s[0], *s[1:]), d) for s, d in zero_shapes
        ]
        out_arrs = sharded(*concat_in, *concat_zeros)
        return [
            {name: np.asarray(out_arrs[i]).reshape(
                NCORES, *out_avals[i].shape)[c]
             for i, name in enumerate(out_names)}
            for c in range(NCORES)
        ]

    run.sharded = sharded
    run.in_names = in_names
    run.out_names = out_names
    run.zero_shapes = zero_shapes
    run.out_avals = out_avals
    run.mesh = mesh
    return run


def _get_runner(loop_n=None):
    key = ("runner", loop_n)
    if key not in _CACHE:
        _CACHE[key] = _make_runner(_get_program(loop_n))
    return _CACHE[key]


def _prep_inputs(inputs):
    keys = np.asarray(inputs["keys"], dtype=np.float32)
    dec = np.asarray(inputs["decoder_state"], dtype=np.float32)
    Ws_w = np.asarray(inputs["Ws_w"], dtype=np.float32)
    Ws_b = np.asarray(inputs["Ws_b"], dtype=np.float32)
    Wa_w = np.asarray(inputs["Wa_w"], dtype=np.float32)
    Wa_b = np.asarray(inputs["Wa_b"], dtype=np.float32)
    Ua_w = np.asarray(inputs["Ua_w"], dtype=np.float32)
    Ua_b = np.asarray(inputs["Ua_b"], dtype=np.float32)
    Va_w = np.asarray(inputs["Va_w"], dtype=np.float32)
    t = int(np.asarray(inputs["t"]))

    query = dec @ Ws_w.T + Ws_b if t == 0 else dec
    q = query @ Wa_w.T + Wa_b                      # [B, DH]
    qb = q + Ua_b[None, :]                         # fold Ua bias into ACT bias

    uaT = np.ascontiguousarray(Ua_w.T).astype(_nbf)          # [DH(d), DH(e)]
    vaT = np.ascontiguousarray(
        Va_w[0].reshape(EC, P).T).astype(_nbf)               # [128, 16]

    keys_bf = keys.astype(_nbf)
    in_maps = []
    for c in range(NCORES):
        bs = slice(c * BPC, (c + 1) * BPC)
        keysT_c = np.ascontiguousarray(
            keys_bf[bs].reshape(BPC * T, DH).T)              # [DH, BPC*T]
        # qbT[p, e*BPC + b] = qb[b, e*128 + p]
        qbT_c = np.ascontiguousarray(
            qb[bs].reshape(BPC, EC, P).transpose(2, 1, 0).reshape(P, EC * BPC))
        in_maps.append({
            "keysT": keysT_c,
            "uaT": uaT,
            "qbT": qbT_c.astype(np.float32),
            "vaT": vaT,
        })
    return in_maps


def _assemble(results):
    av = np.concatenate([results[c]["av_out"] for c in range(NCORES)], axis=0)
    attn = np.concatenate([results[c]["attn_out"] for c in range(NCORES)],
                          axis=0)
    return av.astype(np.float32), attn[:, :, None].astype(np.float32)


def kernel(**inputs):
    in_maps = _prep_inputs(inputs)
    run = _get_runner()
    return _assemble(run(in_maps))


# revision 22
# speedup vs baseline: 1.1384x; 1.1384x over previous
"""Bahdanau attention Trainium2 kernel (8 NeuronCores, data-parallel on batch).

Reference computation (B=32, T=512, DH=2048, fp32):
    query = dec @ Ws_w.T + Ws_b          (t == 0)
    q     = query @ Wa_w.T + Wa_b                     # [B, DH]   (tiny -> host)
    pk    = keys @ Ua_w.T + Ua_b                      # [B, T, DH] (the 137 GFLOP matmul)
    e     = tanh(q[:, None, :] + pk)
    scores= e . Va                                    # [B, T]  (+Va_b: softmax-invariant, dropped)
    attn  = softmax(scores, axis=1)
    av    = attn @ keys                               # [B, DH]

Device strategy per core (4 batches/core), ~330us/core measured:
    - keys^T [DH, 4*T] and Ua^T [DH, DH] fed transposed+bf16 from host
      (contraction dim on partitions; PE needs both operands K-major).
    - main matmul: out[e_chunk(128), t(512)] accumulating 16 k-chunks in PSUM.
    - ACT: tanh(psum + bias) fused, bias = (q+Ua_b)^T per-partition column.
    - scores: PE matmuls lhsT=Va chunk [128,1] accumulated over 16 e-chunks.
    - softmax on [1, 512]: DVE max(negate) -> ACT exp(bias=-max, accum=sum)
      -> DVE reciprocal -> DVE scale.
    - replicate attn to 128 partitions via PE (ones[1,128]^T @ attn[1,512]).
    - av: DVE multiply+reduce over resident keysT tiles -> av^T [128,16],
      written to DRAM directly with a transposed (strided) DMA pattern.

HW-measured notes (loop-timing A/B on the 8-core axon TRN2):
    - tensor_tensor_reduce and gpsimd.partition_broadcast crash / are slow
      in this neuronxcc backend -- avoided.
    - interleaving score matmuls or k-outer interleaved PSUM accumulation
      groups into the main stream measured much SLOWER than clean e-major
      emission (+40-60us) despite cost-model predictions.
    - eliminating the tail PE-transpose (direct strided av DMA) saved ~14us
      by removing a strict-FIFO PE stall on the DVE av chain.
"""

import numpy as np
import ml_dtypes

import concourse.bacc as bacc
import concourse.mybir as mybir
import concourse.tile as tile
from concourse.masks import make_identity

B, T, DH = 32, 512, 2048
NCORES = 8
BPC = B // NCORES          # batches per core
P = 128
KC = DH // P               # contraction chunks
EC = DH // P               # output-feature chunks

F32 = mybir.dt.float32
BF16 = mybir.dt.bfloat16
MM_DT = BF16               # dtype of the big matmuls (bf16: full PE rate)

WARMUP_TABLES = True       # preload ACT Tanh/Exp tables during DMA prologue
PROLOGUE_SPLIT = 0         # batch-0 k-outer prologue split: measured WORSE on HW
INTERLEAVE_VA = False      # interleaved score matmuls: measured WORSE on HW
TAIL_MODE = "full"         # "tanh_only" strips scores/softmax/av (perf probe)
RHS_SPLIT = 1              # split moving dim into chunks (perf probe)
PS_PK_BUFS = 4             # main psum pool depth
STATIC_INPUTS = False      # perf probe: hoist all input DMAs out of the loop
TAIL_IMPL = "direct"       # "defer" | "direct" | "bcast" -- av/rep tail strategy
UA_SPLIT = 1               # column-split Ua DMA: measured neutral/worse on HW

_nbf = ml_dtypes.bfloat16


def build_program(loop_n=None):
    """Build the per-core Bass program. loop_n wraps the body in a hardware
    loop (timing harness only); None = single-shot (grading path)."""
    nc = bacc.Bacc("TRN2", target_bir_lowering=False, debug=False,
                   num_devices=NCORES)

    keysT = nc.dram_tensor("keysT", [DH, BPC * T], MM_DT, kind="ExternalInput")
    uaT = nc.dram_tensor("uaT", [DH, DH], MM_DT, kind="ExternalInput")
    qbT = nc.dram_tensor("qbT", [P, EC * BPC], F32, kind="ExternalInput")
    vaT = nc.dram_tensor("vaT", [P, EC], MM_DT, kind="ExternalInput")
    av_out = nc.dram_tensor("av_out", [BPC, DH], F32, kind="ExternalOutput")
    attn_out = nc.dram_tensor("attn_out", [BPC, T], F32, kind="ExternalOutput")

    av_dram = av_out.ap().rearrange("b (c p) -> b c p", c=EC)  # [BPC, 16, 128]
    # transposed view for writing av^T [128,16] straight to DRAM
    av_dram_t = av_out.ap().rearrange("b (c p) -> b p c", c=EC)  # [BPC,128,16]

    with tile.TileContext(nc) as tc:
        with (
            tc.tile_pool(name="const", bufs=1) as const_pool,
            tc.tile_pool(name="ua", bufs=KC * UA_SPLIT) as ua_pool,
            tc.tile_pool(name="kt", bufs=BPC * KC if STATIC_INPUTS else 2 * KC) as kt_pool,
            tc.tile_pool(name="th", bufs=KC + 4) as th_pool,
            tc.tile_pool(name="small", bufs=8) as small_pool,
            tc.tile_pool(name="scr", bufs=2) as scr_pool,
            tc.tile_pool(name="ps_pk", bufs=max(PS_PK_BUFS, PROLOGUE_SPLIT),
                         space="PSUM") as ps_pk_pool,
            tc.tile_pool(name="ps_sc", bufs=1, space="PSUM") as ps_sc_pool,
            tc.tile_pool(name="ps_rep", bufs=1, space="PSUM") as ps_rep_pool,
            tc.tile_pool(name="ps_av", bufs=1, space="PSUM") as ps_av_pool,
        ):
            identity = const_pool.tile([P, P], F32)
            make_identity(nc, identity[:, :])
            ones = const_pool.tile([1, P], F32)
            nc.vector.memset(ones[:, :], 1.0)

            def load_inputs():
                # ua_sb[j][k]: Ua^T[k*128:(k+1)*128, j*CW:(j+1)*CW].
                # Column-split (UA_SPLIT) orders the stream so the first
                # e-chunks' weights + batch-0 keys land first and the PE
                # starts ~3MB into the prologue instead of 10MB.
                CW = DH // UA_SPLIT
                ua_sb = [[None] * KC for _ in range(UA_SPLIT)]
                kt_all = []
                for k in range(KC):
                    t_ = ua_pool.tile([P, CW], MM_DT, tag="ua",
                                      name=f"ua0_{k}")
                    nc.sync.dma_start(t_[:, :],
                                      uaT.ap()[k * P:(k + 1) * P, 0:CW])
                    ua_sb[0][k] = t_
                    t_ = kt_pool.tile([P, T], MM_DT, tag="kt", name=f"kt0_{k}")
                    nc.sync.dma_start(t_[:, :],
                                      keysT.ap()[k * P:(k + 1) * P, 0:T])
                    kt_all.append(t_)
                qb_sb = small_pool.tile([P, EC * BPC], F32, tag="qb")
                nc.sync.dma_start(qb_sb[:, :], qbT.ap())
                va_sb = small_pool.tile([P, EC], MM_DT, tag="va")
                nc.sync.dma_start(va_sb[:, :], vaT.ap())
                for j in range(1, UA_SPLIT):
                    for k in range(KC):
                        t_ = ua_pool.tile([P, CW], MM_DT, tag="ua",
                                          name=f"ua{j}_{k}")
                        nc.sync.dma_start(
                            t_[:, :],
                            uaT.ap()[k * P:(k + 1) * P,
                                     j * CW:(j + 1) * CW])
                        ua_sb[j][k] = t_
                return ua_sb, kt_all, qb_sb, va_sb

            def load_kt(b):
                kt = []
                for k in range(KC):
                    t_ = kt_pool.tile([P, T], MM_DT, tag="kt",
                                      name=f"kt{b}_{k}")
                    nc.sync.dma_start(
                        t_[:, :],
                        keysT.ap()[k * P:(k + 1) * P, b * T:(b + 1) * T])
                    kt.append(t_)
                return kt

            static = [None]

            def body():
                if STATIC_INPUTS:
                    ua_sb, kt_all, qb_sb, va_sb = static[0]
                else:
                    ua0, kt0, qb0, va0 = load_inputs()
                    ua_sb, kt_all, qb_sb, va_sb = ua0, [kt0], qb0, va0
                if WARMUP_TABLES:
                    # touch Tanh/Exp so ACT table loads overlap the DMA
                    # prologue instead of stalling the first real activation
                    w_ = small_pool.tile([1, 2], F32, tag="warm")
                    nc.vector.memset(w_[:, :], 0.0)
                    nc.scalar.activation(w_[:, :], w_[:, :],
                                         mybir.ActivationFunctionType.Tanh)
                    nc.scalar.activation(w_[:, :], w_[:, :],
                                         mybir.ActivationFunctionType.Exp)

                EPJ = EC // UA_SPLIT   # e-chunks per ua column split
                def ua_lhs(e, k):
                    j, r = divmod(e, EPJ)
                    return ua_sb[j][k][:, r * P:(r + 1) * P]

                pending = []
                for b in range(BPC):
                    # keys^T tiles for this batch: [128(d), 512(t)] x KC
                    if b < len(kt_all) and kt_all[b] is not None:
                        kt = kt_all[b]
                    else:
                        kt = load_kt(b)

                    th = [None] * EC
                    ps_sc = ps_sc_pool.tile([1, T], F32, tag="ps_sc")
                    n_va = [0]

                    def emit_tanh(e, ps, b=b, th=th):
                        t_ = th_pool.tile([P, T], MM_DT, tag="th")
                        nc.scalar.activation(
                            t_[:, :], ps[:, :],
                            mybir.ActivationFunctionType.Tanh,
                            bias=qb_sb[:, e * BPC + b:e * BPC + b + 1],
                            scale=1.0)
                        th[e] = t_

                    def emit_va(e, ps_sc=ps_sc, th=th, n_va=n_va):
                        # scores[1,512] += Va_chunk[e]^T @ th[e]
                        nc.tensor.matmul(
                            ps_sc[:, :], va_sb[:, e:e + 1], th[e][:, :],
                            start=(n_va[0] == 0), stop=(n_va[0] == EC - 1))
                        n_va[0] += 1

                    # pk[e] = sum_k UaT[k,e]^T @ keysT[k]  -> tanh(+bias)
                    pa = PROLOGUE_SPLIT if b == 0 else 0
                    if pa:
                        # k-outer over the first `pa` e-chunks: the PE chews
                        # each (ua[k], kt0[k]) pair as it lands instead of
                        # stalling until the whole 10MB prologue arrives
                        pss = [ps_pk_pool.tile([P, T], F32, tag="ps_pk",
                                               name=f"pss{i}")
                               for i in range(pa)]
                        for k in range(KC):
                            for e in range(pa):
                                nc.tensor.matmul(
                                    pss[e][:, :],
                                    ua_lhs(e, k),
                                    kt[k][:, :],
                                    start=(k == 0), stop=(k == KC - 1))
                        for e in range(pa):
                            emit_tanh(e, pss[e])

                    va_queue = list(range(pa))
                    NS = T // RHS_SPLIT
                    for e in range(pa, EC):
                        ps = ps_pk_pool.tile([P, T], F32, tag="ps_pk")
                        for k in range(KC):
                            for h in range(RHS_SPLIT):
                                nc.tensor.matmul(
                                    ps[:, h * NS:(h + 1) * NS],
                                    ua_lhs(e, k),
                                    kt[k][:, h * NS:(h + 1) * NS],
                                    start=(k == 0), stop=(k == KC - 1))
                        emit_tanh(e, ps)
                        # PE queue is strict FIFO: the previous batch's
                        # rep/transpose matmuls would stall PE on the DVE/
                        # softmax chain if emitted in their own batch, so
                        # they are spliced here, after their deps are long
                        # satisfied.
                        if pending and e == 1:
                            pending[0][0]()
                        if pending and e == 6:
                            pending.pop(0)[1]()
                        if INTERLEAVE_VA:
                            # previous chunk's tanh finished during this
                            # group's matmuls; fold it into scores now
                            while va_queue:
                                emit_va(va_queue.pop(0))
                            va_queue.append(e)
                        else:
                            va_queue.append(e)
                    if TAIL_MODE == "tanh_only":
                        # perf probe: main matmuls + tanh only; dump one tanh
                        # tile so nothing is dead-code-eliminated
                        dump = small_pool.tile([1, T], F32, tag="attn")
                        nc.vector.tensor_copy(dump[:, :], th[EC - 1][0:1, :])
                        nc.sync.dma_start(attn_out.ap()[b:b + 1, :],
                                          dump[:, :])
                        continue

                    for e in va_queue:
                        emit_va(e)

                    # softmax over the 512 scores (single partition)
                    negmax = small_pool.tile([1, 1], F32, tag="negmax")
                    nc.vector.tensor_reduce(
                        negmax[:, :], ps_sc[:, :],
                        axis=mybir.AxisListType.X, op=mybir.AluOpType.max,
                        negate=True)
                    exp_sb = small_pool.tile([1, T], F32, tag="exp")
                    sumexp = small_pool.tile([1, 1], F32, tag="sumexp")
                    nc.scalar.activation(
                        exp_sb[:, :], ps_sc[:, :],
                        mybir.ActivationFunctionType.Exp,
                        bias=negmax[:, :], scale=1.0,
                        accum_out=sumexp[:, :])
                    inv = small_pool.tile([1, 1], F32, tag="inv")
                    nc.vector.reciprocal(inv[:, :], sumexp[:, :])
                    attn_sb = small_pool.tile([1, T], F32, tag="attn")
                    nc.vector.tensor_scalar_mul(
                        attn_sb[:, :], exp_sb[:, :], inv[:, :])
                    nc.sync.dma_start(attn_out.ap()[b:b + 1, :], attn_sb[:, :])

                    if TAIL_IMPL in ("direct", "bcast"):
                        if TAIL_IMPL == "bcast":
                            rep_sb = scr_pool.tile([P, T], F32, tag="rep_sb",
                                                   name=f"rep_sb{b}")
                            nc.gpsimd.partition_broadcast(
                                rep_sb[:, :], attn_sb[:, :], channels=P)
                            rep_ap = rep_sb
                        else:
                            rep_ps = ps_rep_pool.tile([P, T], F32,
                                                      tag="ps_rep",
                                                      name=f"ps_rep{b}")
                            nc.tensor.matmul(
                                rep_ps[:, :], ones[:, :], attn_sb[:, :],
                                start=True, stop=True)
                            rep_ap = rep_ps
                        av_t = small_pool.tile([P, EC], F32, tag="av_t",
                                               name=f"av_t{b}")
                        for d in range(KC):
                            scr = scr_pool.tile([P, T], F32, tag="scr",
                                                name=f"scr{b}_{d}")
                            nc.vector.tensor_tensor(
                                out=scr[:, :], in0=kt[d][:, :],
                                in1=rep_ap[:, :], op=mybir.AluOpType.mult)
                            nc.vector.reduce_sum(
                                av_t[:, d:d + 1], scr[:, :],
                                axis=mybir.AxisListType.X)
                        nc.sync.dma_start(av_dram_t[b], av_t[:, :])
                        continue

                    def make_deferred(b=b, kt=kt, attn_sb=attn_sb):
                        st = {}

                        def do_rep():
                            # replicate attn across partitions: ones^T @ attn
                            ps_rep = ps_rep_pool.tile(
                                [P, T], F32, tag="ps_rep", name=f"ps_rep{b}")
                            nc.tensor.matmul(
                                ps_rep[:, :], ones[:, :], attn_sb[:, :],
                                start=True, stop=True)
                            # av^T[:, d] = sum_t keysT[d][:, t] * attn[t]
                            # (tensor_tensor_reduce is broken in this
                            #  backend -- separate multiply + reduce)
                            av_t = small_pool.tile([P, EC], F32, tag="av_t",
                                                   name=f"av_t{b}")
                            for d in range(KC):
                                scr = scr_pool.tile([P, T], F32, tag="scr",
                                                    name=f"scr{b}_{d}")
                                nc.vector.tensor_tensor(
                                    out=scr[:, :], in0=kt[d][:, :],
                                    in1=ps_rep[:, :],
                                    op=mybir.AluOpType.mult)
                                nc.vector.reduce_sum(
                                    av_t[:, d:d + 1], scr[:, :],
                                    axis=mybir.AxisListType.X)
                            st["av_t"] = av_t

                        def do_tr():
                            # transpose [128,16] -> [16,128], copy out
                            ps_av = ps_av_pool.tile(
                                [EC, P], F32, tag="ps_av", name=f"ps_av{b}")
                            nc.tensor.transpose(ps_av[:, :],
                                                st["av_t"][:, :],
                                                identity[:, :])
                            av_sb = small_pool.tile([EC, P], F32,
                                                    tag="av_sb",
                                                    name=f"av_sb{b}")
                            nc.vector.tensor_copy(av_sb[:, :], ps_av[:, :])
                            nc.sync.dma_start(av_dram[b], av_sb[:, :])

                        return do_rep, do_tr

                    pending.append(make_deferred())

                # tail: flush deferred work of the final batch(es)
                for do_rep, do_tr in pending:
                    do_rep()
                    do_tr()
                pending.clear()

            if STATIC_INPUTS:
                ua0, kt0, qb0, va0 = load_inputs()
                kts = [kt0] + [load_kt(b) for b in range(1, BPC)]
                static[0] = (ua0, kts, qb0, va0)
            if loop_n is None:
                body()
            else:
                with tc.For_i(0, loop_n, 1,
                              hint_engines=(mybir.EngineType.PE,)):
                    body()

    nc.compile()
    return nc


_CACHE = {}


def _get_program(loop_n=None):
    key = ("prog", loop_n)
    if key not in _CACHE:
        _CACHE[key] = build_program(loop_n)
    return _CACHE[key]


def _make_runner(nc):
    """Build a cached jitted SPMD executor for `nc` (the library's
    run_bass_kernel_spmd re-traces jax.jit on every call, which costs ~3s;
    tracing once makes repeat calls cheap)."""
    import jax
    from jax.experimental.shard_map import shard_map
    from jax.sharding import Mesh, PartitionSpec
    from concourse.bass2jax import (_bass_exec_p, install_neuronx_cc_hook,
                                    partition_id_tensor)

    install_neuronx_cc_hook()

    partition_name = (nc.partition_id_tensor.name
                      if nc.partition_id_tensor else None)
    in_names, out_names, out_avals, zero_shapes = [], [], [], []
    for alloc in nc.m.functions[0].allocations:
        if not isinstance(alloc, mybir.MemoryLocationSet):
            continue
        name = alloc.memorylocations[0].name
        if alloc.kind == "ExternalInput":
            if name != partition_name:
                in_names.append(name)
        elif alloc.kind == "ExternalOutput":
            shape = tuple(alloc.tensor_shape)
            dtype = mybir.dt.np(alloc.dtype)
            out_names.append(name)
            out_avals.append(jax.core.ShapedArray(shape, dtype))
            zero_shapes.append((shape, dtype))
    n_params = len(in_names)
    all_names = list(in_names + out_names)
    if partition_name is not None:
        all_names.append(partition_name)
    all_names = tuple(all_names)
    donate = tuple(range(n_params, n_params + len(out_names)))

    def _body(*args):
        operands = list(args)
        if partition_name is not None:
            operands.append(partition_id_tensor())
        outs = _bass_exec_p.bind(
            *operands,
            out_avals=tuple(out_avals),
            in_names=all_names,
            out_names=tuple(out_names),
            lowering_input_output_aliases=(),
            sim_require_finite=True,
            sim_require_nnan=True,
            nc=nc,
        )
        return tuple(outs)

    devices = jax.devices()[:NCORES]
    mesh = Mesh(np.asarray(devices), ("core",))
    nio = n_params + len(out_names)
    sharded = jax.jit(
        shard_map(_body, mesh=mesh, in_specs=(PartitionSpec("core"),) * nio,
                  out_specs=(PartitionSpec("core"),) * len(out_names),
                  check_rep=False),
        donate_argnums=donate, keep_unused=True)

    def run(in_maps):
        concat_in = [
            np.concatenate([np.asarray(m[name]) for m in in_maps], axis=0)
            for name in in_names
        ]
        concat_zeros = [
            np.zeros((NCORES * s[0], *s[1:]), d) for s, d in zero_shapes
        ]
        out_arrs = sharded(*concat_in, *concat_zeros)
        return [
            {name: np.asarray(out_arrs[i]).reshape(
                NCORES, *out_avals[i].shape)[c]
             for i, name in enumerate(out_names)}
            for c in range(NCORES)
        ]

    run.sharded = sharded
    run.in_names = in_names
    run.out_names = out_names
    run.zero_shapes = zero_shapes
    run.out_avals = out_avals
    run.mesh = mesh
    return run


def _get_runner(loop_n=None):
    key = ("runner", loop_n)
    if key not in _CACHE:
        _CACHE[key] = _make_runner(_get_program(loop_n))
    return _CACHE[key]


def _prep_inputs(inputs):
    keys = np.asarray(inputs["keys"], dtype=np.float32)
    dec = np.asarray(inputs["decoder_state"], dtype=np.float32)
    Ws_w = np.asarray(inputs["Ws_w"], dtype=np.float32)
    Ws_b = np.asarray(inputs["Ws_b"], dtype=np.float32)
    Wa_w = np.asarray(inputs["Wa_w"], dtype=np.float32)
    Wa_b = np.asarray(inputs["Wa_b"], dtype=np.float32)
    Ua_w = np.asarray(inputs["Ua_w"], dtype=np.float32)
    Ua_b = np.asarray(inputs["Ua_b"], dtype=np.float32)
    Va_w = np.asarray(inputs["Va_w"], dtype=np.float32)
    t = int(np.asarray(inputs["t"]))

    query = dec @ Ws_w.T + Ws_b if t == 0 else dec
    q = query @ Wa_w.T + Wa_b                      # [B, DH]
    qb = q + Ua_b[None, :]                         # fold Ua bias into ACT bias

    uaT = np.ascontiguousarray(Ua_w.T).astype(_nbf)          # [DH(d), DH(e)]
    vaT = np.ascontiguousarray(
        Va_w[0].reshape(EC, P).T).astype(_nbf)               # [128, 16]

    keys_bf = keys.astype(_nbf)
    in_maps = []
    for c in range(NCORES):
        bs = slice(c * BPC, (c + 1) * BPC)
        keysT_c = np.ascontiguousarray(
            keys_bf[bs].reshape(BPC * T, DH).T)              # [DH, BPC*T]
        # qbT[p, e*BPC + b] = qb[b, e*128 + p]
        qbT_c = np.ascontiguousarray(
            qb[bs].reshape(BPC, EC, P).transpose(2, 1, 0).reshape(P, EC * BPC))
        in_maps.append({
            "keysT": keysT_c,
            "uaT": uaT,
            "qbT": qbT_c.astype(np.float32),
            "vaT": vaT,
        })
    return in_maps


def _assemble(results):
    av = np.concatenate([results[c]["av_out"] for c in range(NCORES)], axis=0)
    attn = np.concatenate([results[c]["attn_out"] for c in range(NCORES)],
                          axis=0)
    return av.astype(np.float32), attn[:, :, None].astype(np.float32)


def kernel(**inputs):
    in_maps = _prep_inputs(inputs)
    run = _get_runner()
    return _assemble(run(in_maps))
